# revision 43
# baseline (speedup 1.0000x reference)
"""Trainium2 Bass kernel for paged causal self-attention (GQA + YaRN rope).

Sharding: tensor-parallel over heads. Core c (of 8) owns kv-head c and
q-heads 2c, 2c+1 for both batches. Each core computes a partial output
y_c = attn_c @ Wo_c.T over its 256 channels; the host sums the 8 partials.

The reference's scatter of new K/V into the pools is dead code w.r.t. the
returned output (slot_map is a permutation, so gathered past slots are
disjoint from the scattered new slots); new K/V are consumed directly from
SBUF and only the past 1024 slots per batch are gathered via indirect DMA.

v2: all matmul operands in bf16 (host pre-casts inputs; intermediates are
written bf16 by their producing ops, so no rounding-copy traffic). Weights
are pre-permuted on host into dense [128, ...] layouts for single large
DMAs. The softmax denominator is accumulated on the PE via indicator-column
matmuls into a [2, 512] PSUM tile; 1/x is computed as Exp(-Ln(x)) on the
scalar engine (same activation table set as the attention exp). RoPE keeps
an fp32r path for the rotate-halves matmul, fp32 cos/sin tables, and only
rounds to bf16 on the final write.
"""

import sys

sys.path.insert(0, "/opt/trn_rl_repo")

import numpy as np
import ml_dtypes

import concourse.bacc as bacc
import concourse.tile as tile
from concourse import mybir
from concourse.bass import IndirectOffsetOnAxis
from concourse.bass_utils import run_bass_kernel_spmd

F32 = mybir.dt.float32
F32R = mybir.dt.float32r
BF16 = mybir.dt.bfloat16
I32 = mybir.dt.int32
EXP = mybir.ActivationFunctionType.Exp
LN = mybir.ActivationFunctionType.Ln
NPBF = ml_dtypes.bfloat16

B, T, PAST = 2, 1024, 1024
H, HKV, D = 16, 8, 128
G = H // HKV            # q heads per kv head
C = H * D               # 2048
TOTAL = PAST + T        # 2048
NB = B * T              # 2048 flattened tokens
NCORES = 8
P = 128
TB = 512                # token block
NEG = -1.0e30


def _f(ap):
    return ap.bitcast(F32)


def _emit(tc, io):
    nc = tc.nc
    (xT, wq, wk, wv, wo, kp, vp, gidx, cosq, sinq, cosk, sink,
     cmask, rperm, identb, selg, ind2, y) = io

    with (
        tc.tile_pool(name="const", bufs=1) as cp,
        tc.tile_pool(name="persist", bufs=1) as pp,
        tc.tile_pool(name="ysb", bufs=2) as yp,
    ):
        # ---- constants ----
        gidx_t = cp.tile([P, 2 * 8], I32)
        nc.sync.dma_start(gidx_t[:], gidx[:])
        sel_s = cp.tile([2, G * P], F32)
        nc.sync.dma_start(sel_s[:], selg[:])
        sel_t = cp.tile([2, G * P], F32R)
        nc.vector.tensor_copy(sel_t[:], sel_s[:])
        cmask_t = cp.tile([P, 4, TB], F32)   # DMA deferred until after x loads
        identb_t = cp.tile([P, P], BF16)
        nc.sync.dma_start(identb_t[:], identb[:])
        rperm_s = cp.tile([P, P], F32)
        nc.sync.dma_start(rperm_s[:], rperm[:])
        rperm_t = cp.tile([P, P], F32R)
        nc.vector.tensor_copy(rperm_t[:], rperm_s[:])

        # [:, g, :] = 128-col indicator (col g ones): full-width stationary
        # keeps the fast-weight-load path on for den matmuls
        ind2_t = cp.tile([P, 2, P], BF16)
        nc.sync.dma_start(ind2_t[:], ind2[:])

        # ---- persistent activations (bf16) ----
        qT0 = pp.tile([P, NB], BF16)      # q head 2c,   [d, token]
        qT1 = pp.tile([P, NB], BF16)      # q head 2c+1
        kT_new = pp.tile([P, NB], BF16)   # new keys,    [d, token]
        v_nat = pp.tile([P, B, 8, P], BF16)    # new values, [t%128, b, chunk, d]
        kT_past = pp.tile([P, B, 8, P], BF16)  # past keys,  [d, b, chunk, s%128]
        wo_t = pp.tile([P, G, C], BF16)   # [d-in-head, g, out-ch]
        kg = [None, None]
        vg = [None, None]

        # ================= phase 1: projections + rope =================
        with (
            tc.tile_pool(name="tabs", bufs=1) as tbp,
            tc.tile_pool(name="wts", bufs=1) as wp,
            tc.tile_pool(name="xt", bufs=1) as xp,
            tc.tile_pool(name="rope", bufs=3) as rp,
            tc.tile_pool(name="pproj", bufs=1, space="PSUM") as pjp,
            tc.tile_pool(name="prope", bufs=2, space="PSUM") as rpp,
            tc.tile_pool(name="ptr", bufs=2, space="PSUM") as trp,
        ):
            # weights: single dense DMAs (host pre-permuted layouts)
            wq_t = wp.tile([P, 16, G * P], BF16)
            nc.sync.dma_start(wq_t[:], wq[:])
            wk_t = wp.tile([P, 16, P], BF16)
            nc.sync.dma_start(wk_t[:], wk[:])
            wv_t = wp.tile([P, 16, P], BF16)
            nc.sync.dma_start(wv_t[:], wv[:])

            # x rows staged once: 16 tiles of [P, NB] (one per contraction
            # chunk), each loaded in two halves so block 0 can start early
            xts = []
            for kc in range(16):
                xt = xp.tile([P, NB], BF16, name=f"xt{kc}", tag=f"xt{kc}")
                nc.sync.dma_start(xt[:, 0:T], xT[kc * P:(kc + 1) * P, 0:T])
                xts.append(xt)

            # rope tables (phase-1 only, fp32); needed only after block 0's
            # projection matmuls, so they queue behind the first x halves
            cosq_t = tbp.tile([P, T], F32)
            nc.sync.dma_start(cosq_t[:], cosq[:])
            sinq_t = tbp.tile([P, T], F32)
            nc.sync.dma_start(sinq_t[:], sinq[:])
            cosk_t = tbp.tile([P, T], F32)
            nc.sync.dma_start(cosk_t[:], cosk[:])
            sink_t = tbp.tile([P, T], F32)
            nc.sync.dma_start(sink_t[:], sink[:])

            for kc in range(16):
                nc.sync.dma_start(xts[kc][:, T:NB], xT[kc * P:(kc + 1) * P, T:NB])
            nc.sync.dma_start(cmask_t[:], cmask[:])

            # past K/V gather (bf16 rows; needed only by phase-1 tail)
            for b in range(B):
                kg[b] = pp.tile([P, 8, P], BF16, name=f"kg{b}", tag=f"kg{b}")
                vg[b] = pp.tile([P, 8, P], BF16, name=f"vg{b}", tag=f"vg{b}")
                for j in range(8):
                    # [P,1]-index gathers: the multi-column offset-AP form
                    # miscomputes on hardware
                    nc.gpsimd.indirect_dma_start(
                        out=kg[b][:, j, :],
                        out_offset=None,
                        in_=kp[:, :],
                        in_offset=IndirectOffsetOnAxis(
                            ap=gidx_t[:, 8 * b + j:8 * b + j + 1], axis=0),
                    )
                    nc.gpsimd.indirect_dma_start(
                        out=vg[b][:, j, :],
                        out_offset=None,
                        in_=vp[:, :],
                        in_offset=IndirectOffsetOnAxis(
                            ap=gidx_t[:, 8 * b + j:8 * b + j + 1], axis=0),
                    )

            wo_dma_done = False
            for tb in range(NB // TB):           # 4 token blocks of 512
                n0 = tb * TB
                b = tb // 2
                tpos = (tb % 2) * TB             # position-in-batch of block start

                q0p = pjp.tile([P, TB], F32, name="q0p", tag="q0")
                q1p = pjp.tile([P, TB], F32, name="q1p", tag="q1")
                kkp = pjp.tile([P, TB], F32, name="kkp", tag="kk")
                vvp = pjp.tile([P, TB], F32, name="vvp", tag="vv")
                for kc in range(16):
                    xt = xts[kc][:, n0:n0 + TB]
                    st = (kc == 0)
                    sp = (kc == 15)
                    nc.tensor.matmul(q0p[:], wq_t[:, kc, 0:P], xt, start=st, stop=sp)
                    nc.tensor.matmul(q1p[:], wq_t[:, kc, P:2 * P], xt, start=st, stop=sp)
                    nc.tensor.matmul(kkp[:], wk_t[:, kc, :], xt, start=st, stop=sp)
                    nc.tensor.matmul(vvp[:], wv_t[:, kc, :], xt, start=st, stop=sp)

                # rope for q0, q1, k: dst = raw*cos + rot(raw)*sin, bf16 on write
                for src, dst, ct, stt in (
                    (q0p, qT0, cosq_t, sinq_t),
                    (q1p, qT1, cosq_t, sinq_t),
                    (kkp, kT_new, cosk_t, sink_t),
                ):
                    raw = rp.tile([P, TB], F32R, name="raw", tag="raw")
                    nc.vector.tensor_copy(raw[:], src[:])
                    rot = rpp.tile([P, TB], F32, name="rot", tag="rot")
                    nc.tensor.matmul(rot[:], rperm_t[:], raw[:], start=True, stop=True)
                    t1 = rp.tile([P, TB], F32, name="t1", tag="t1")
                    nc.vector.tensor_mul(t1[:], _f(raw[:]), ct[:, tpos:tpos + TB])
                    t2 = rp.tile([P, TB], F32, name="t2", tag="t2")
                    nc.vector.tensor_mul(t2[:], rot[:], stt[:, tpos:tpos + TB])
                    nc.vector.tensor_add(dst[:, n0:n0 + TB], t1[:], t2[:])

                # v: no rope; transpose [d, t] -> [t, d] in 128-chunks
                vraw = rp.tile([P, TB], BF16, name="vraw", tag="vraw")
                nc.scalar.copy(vraw[:], vvp[:])
                for j4 in range(TB // P):
                    vt = trp.tile([P, P], BF16, name="vt", tag="tr")
                    nc.tensor.transpose(vt[:], vraw[:, j4 * P:(j4 + 1) * P],
                                        identb_t[:])
                    nc.vector.tensor_copy(v_nat[:, b, (tb % 2) * 4 + j4, :], vt[:])

                if not wo_dma_done:
                    # emit after block 0 so it doesn't delay the first matmuls
                    nc.sync.dma_start(wo_t[:], wo[:])
                    wo_dma_done = True

            # past K transpose: [s, d] -> [d, s]
            for b in range(B):
                for j in range(8):
                    kt = trp.tile([P, P], BF16, name="kt", tag="tr")
                    nc.tensor.transpose(kt[:], kg[b][:, j, :], identb_t[:])
                    nc.vector.tensor_copy(kT_past[:, b, j, :], kt[:])

        # ================= phase 2: attention + output proj =================
        with (
            tc.tile_pool(name="attp", bufs=1) as ap_,
            tc.tile_pool(name="exps", bufs=8) as ep,
            tc.tile_pool(name="sums", bufs=2) as sp_,
            tc.tile_pool(name="avsb", bufs=3) as asp,
            tc.tile_pool(name="pscore", bufs=2, space="PSUM") as scp,
            tc.tile_pool(name="pav", bufs=2, space="PSUM") as avp,
            tc.tile_pool(name="pden", bufs=1, space="PSUM") as dnp,
            tc.tile_pool(name="py", bufs=1, space="PSUM") as pyp,
        ):
            att0 = ap_.tile([P, NB], BF16)    # attention out head 2c, [d, token]
            att1 = ap_.tile([P, NB], BF16)

            # Two-level software pipeline. Within a group: scores+exp for
            # chunk ci, av/den matmuls for chunk ci-LAG (accumulation order
            # is commutative), so the strict-FIFO tensor queue never parks an
            # av behind a fresh exp. Across groups: the normalization tail
            # (waits on the DVE reciprocal chain) and the output projection
            # are emitted in the MIDDLE of the next group's chunk stream so
            # their waits resolve behind already-ready score matmuls.
            LAG = 4
            pending_norm = None
            pending_yproj = None

            def make_norm_tail(den, avs, t0):
                def tail():
                    av_sb = [None, None]
                    for g in range(G):
                        av_sb[g] = asp.tile([P, TB], F32, name=f"avsb{g}",
                                            tag="avsb")
                        nc.vector.tensor_copy(av_sb[g][:], avs[g][:])
                    den_sb = sp_.tile([2, TB], F32, name="den_sb",
                                      tag="den_sb")
                    nc.vector.tensor_copy(den_sb[:], den[0:2, :])
                    rinv_f = sp_.tile([2, TB], F32, name="rinv_f",
                                      tag="rinv_f")
                    nc.vector.reciprocal_approx_fast(rinv_f[:], den_sb[:])
                    rinv = sp_.tile([2, TB], F32R, name="rinv", tag="rinv")
                    nc.vector.tensor_copy(rinv[:], rinv_f[:])
                    for g, att in enumerate((att0, att1)):
                        rbc = pyp.tile([P, TB], F32, name="rbc", tag="ybc")
                        nc.tensor.matmul(rbc[:], sel_t[:, g * P:(g + 1) * P],
                                         rinv[:, :], start=True, stop=True)
                        rbs = sp_.tile([P, TB], F32, name="rbs", tag="rbs")
                        nc.vector.tensor_copy(rbs[:], rbc[:])
                        nc.vector.tensor_mul(att[:, t0:t0 + TB], av_sb[g][:],
                                             rbs[:])
                return tail

            def make_yproj_tail(t0):
                def tail():
                    for tc4 in range(4):
                        tt0 = t0 + tc4 * P
                        ysb = yp.tile([P, 4, TB], BF16, name="ysbt",
                                      tag="ysbt")
                        for cb in range(4):
                            yps = pyp.tile([P, TB], F32, name="yps",
                                           tag="ybc")
                            nc.tensor.matmul(yps[:], att0[:, tt0:tt0 + P],
                                             wo_t[:, 0, cb * TB:(cb + 1) * TB],
                                             start=True, stop=False)
                            nc.tensor.matmul(yps[:], att1[:, tt0:tt0 + P],
                                             wo_t[:, 1, cb * TB:(cb + 1) * TB],
                                             start=False, stop=True)
                            nc.vector.tensor_copy(ysb[:, cb, :], yps[:])
                        nc.sync.dma_start(y[tt0:tt0 + P, :], ysb[:])
                return tail

            for b in range(B):
                for tbq in range(2):             # query block of 512 within batch
                    t0 = b * T + tbq * TB        # global token offset
                    njnew = 4 * tbq + 4
                    nch = 8 + njnew

                    chunks = [(kT_past[:, b, j, :], vg[b][:, j, :], None)
                              for j in range(8)]
                    for j in range(njnew):
                        koff = b * T + j * P
                        ri = j - 4 * tbq
                        chunks.append((kT_new[:, koff:koff + P],
                                       v_nat[:, b, j, :],
                                       ri if ri >= 0 else None))

                    den = dnp.tile([P, TB], F32, name="den", tag="den")
                    avs = [None, None]
                    for g in range(G):
                        avs[g] = avp.tile([P, TB], F32, name=f"av{g}", tag="av")

                    e2s = [None] * nch

                    def emit_scores(ci, chunks=chunks, t0=t0, e2s=e2s):
                        k_ap, v_ap, mri = chunks[ci]
                        c0 = 0 if mri is None else mri * P
                        s2 = scp.tile([P, 2, TB], F32, name="s2", tag="s")
                        for g, qT in enumerate((qT0, qT1)):
                            nc.tensor.matmul(s2[:, g, c0:], k_ap,
                                             qT[:, t0 + c0:t0 + TB],
                                             start=True, stop=True)
                            if mri is not None:
                                # first 128 live cols form a plain diagonal
                                # triangle (== ri=0 mask restricted to P)
                                nc.vector.tensor_add(
                                    s2[:, g, c0:c0 + P], s2[:, g, c0:c0 + P],
                                    cmask_t[:, 0, 0:P])
                        e2 = ep.tile([P, 2, TB], BF16, name="e2", tag="e")
                        nc.scalar.activation(e2[:, :, c0:], s2[:, :, c0:], EXP)
                        e2s[ci] = e2

                    def emit_avden(ci, chunks=chunks, e2s=e2s, den=den,
                                   avs=avs, nch=nch):
                        k_ap, v_ap, mri = chunks[ci]
                        c0 = 0 if mri is None else mri * P
                        e2 = e2s[ci]
                        for g in range(G):
                            nc.tensor.matmul(avs[g][:, c0:], v_ap,
                                             e2[:, g, c0:],
                                             start=(ci == 0), stop=(ci == nch - 1))
                            nc.tensor.matmul(den[:, c0:], ind2_t[:, g, :],
                                             e2[:, g, c0:],
                                             start=(ci == 0 and g == 0),
                                             stop=(ci == nch - 1 and g == 1))

                    for ci in range(nch):
                        emit_scores(ci)
                        if ci == 3 and pending_norm is not None:
                            pending_norm()
                            pending_norm = None
                        if ci == 5 and pending_yproj is not None:
                            pending_yproj()
                            pending_yproj = None
                        if ci >= LAG:
                            emit_avden(ci - LAG)
                    for ci in range(nch - LAG, nch):
                        emit_avden(ci)

                    pending_norm = make_norm_tail(den, avs, t0)
                    pending_yproj = make_yproj_tail(t0)

            pending_norm()
            pending_yproj()


def build_nc():
    nc = bacc.Bacc("TRN2")
    xT = nc.dram_tensor("xT", [C, NB], BF16, kind="ExternalInput")
    wq = nc.dram_tensor("wq", [P, 16, G * P], BF16, kind="ExternalInput")
    wk = nc.dram_tensor("wk", [P, 16, P], BF16, kind="ExternalInput")
    wv = nc.dram_tensor("wv", [P, 16, P], BF16, kind="ExternalInput")
    wo = nc.dram_tensor("wo", [P, G, C], BF16, kind="ExternalInput")
    kp = nc.dram_tensor("kp", [B * TOTAL, D], BF16, kind="ExternalInput")
    vp = nc.dram_tensor("vp", [B * TOTAL, D], BF16, kind="ExternalInput")
    gidx = nc.dram_tensor("gidx", [P, B * 8], I32, kind="ExternalInput")
    cosq = nc.dram_tensor("cosq", [P, T], F32, kind="ExternalInput")
    sinq = nc.dram_tensor("sinq", [P, T], F32, kind="ExternalInput")
    cosk = nc.dram_tensor("cosk", [P, T], F32, kind="ExternalInput")
    sink = nc.dram_tensor("sink", [P, T], F32, kind="ExternalInput")
    cmask = nc.dram_tensor("cmask", [P, 4, TB], F32, kind="ExternalInput")
    rperm = nc.dram_tensor("rperm", [P, P], F32, kind="ExternalInput")
    identb = nc.dram_tensor("identb", [P, P], BF16, kind="ExternalInput")
    selg = nc.dram_tensor("selg", [2, G * P], F32, kind="ExternalInput")
    ind2 = nc.dram_tensor("ind2", [P, 2, P], BF16, kind="ExternalInput")
    y = nc.dram_tensor("y", [NB, C], BF16, kind="ExternalOutput")
    io = (xT, wq, wk, wv, wo, kp, vp, gidx, cosq, sinq, cosk, sink,
          cmask, rperm, identb, selg, ind2, y)
    with nc.allow_low_precision(reason="bf16 operands; fp32r rope path"):
        with tile.TileContext(nc) as tc:
            _emit(tc, io)
    nc.compile()
    return nc


def host_inputs(x, Wq, Wkv, Wo, K_pool, V_pool, slot_map, past_len):
    x = np.asarray(x, dtype=np.float32)
    Wq = np.asarray(Wq, dtype=np.float32)
    Wkv = np.asarray(Wkv, dtype=np.float32)
    Wo = np.asarray(Wo, dtype=np.float32)
    K_pool = np.asarray(K_pool, dtype=np.float32)
    V_pool = np.asarray(V_pool, dtype=np.float32)
    slot_map = np.asarray(slot_map, dtype=np.int32)
    past = int(past_len)
    assert past == PAST, f"kernel hardcodes past_len={PAST}, got {past}"

    xT = np.ascontiguousarray(x.reshape(NB, C).T.astype(NPBF))

    # rope tables; argument arithmetic mirrors the f32 ops of the reference
    idx = np.arange(D // 2, dtype=np.float32)
    inv = np.float32(1.0) / np.float32(10000.0) ** (idx / np.float32(D // 2))
    inv = inv.astype(np.float32)
    t = np.arange(past, past + T, dtype=np.float32)
    freqs = (t[:, None] * inv[None, :]).astype(np.float32)
    emb = np.concatenate([freqs, freqs], axis=1)
    cos = np.cos(emb).astype(np.float32)
    sin = np.sin(emb).astype(np.float32)
    qscale = np.float32(1.0) / np.sqrt(np.float32(D))
    cosqT = np.ascontiguousarray((cos * qscale).T)
    sinqT = np.ascontiguousarray((sin * qscale).T)
    coskT = np.ascontiguousarray(cos.T)
    sinkT = np.ascontiguousarray(sin.T)

    s_i = np.arange(P, dtype=np.int64)[:, None]
    t_i = np.arange(TB, dtype=np.int64)[None, :]
    cm = np.empty((P, 4, TB), np.float32)
    for ri in range(4):
        cm[:, ri, :] = np.where(s_i <= t_i - ri * P, 0.0, NEG)

    gidx = slot_map[:, :past].reshape(B, 8, P).transpose(2, 0, 1).reshape(P, B * 8)
    gidx = np.ascontiguousarray(gidx.astype(np.int32))

    rperm = np.zeros((P, P), np.float32)
    for dd in range(D // 2):
        rperm[dd + D // 2, dd] = -1.0     # rot(q)[d] = -q[d+64] for d < 64
        rperm[dd, dd + D // 2] = 1.0      # rot(q)[d] = q[d-64] for d >= 64
    identb = np.eye(P, dtype=NPBF)
    selg = np.zeros((2, G * P), np.float32)
    selg[0, 0:P] = 1.0
    selg[1, P:2 * P] = 1.0
    ind2 = np.zeros((P, 2, P), np.float32)
    ind2[:, 0, 0] = 1.0
    ind2[:, 1, 1] = 1.0
    ind2 = ind2.astype(NPBF)

    def wslice_q(c):
        # [C, 256] -> [128, 16, 256] with element [p, kc, m] = WqT[kc*128+p, m]
        wt = Wq[G * D * c:G * D * (c + 1), :].T            # [C, 256]
        return np.ascontiguousarray(
            wt.reshape(16, P, G * P).transpose(1, 0, 2).astype(NPBF))

    def wslice_kv(row0, c):
        wt = Wkv[row0 + D * c:row0 + D * (c + 1), :].T     # [C, 128]
        return np.ascontiguousarray(
            wt.reshape(16, P, P).transpose(1, 0, 2).astype(NPBF))

    def wslice_o(c):
        wt = Wo[:, G * D * c:G * D * (c + 1)].T            # [512, C]
        return np.ascontiguousarray(
            wt.reshape(G, P, C).transpose(1, 0, 2).astype(NPBF))

    in_maps = []
    for c in range(NCORES):
        in_maps.append({
            "xT": xT,
            "wq": wslice_q(c),
            "wk": wslice_kv(0, c),
            "wv": wslice_kv(HKV * D, c),
            "wo": wslice_o(c),
            "kp": np.ascontiguousarray(K_pool[:, c, :].astype(NPBF)),
            "vp": np.ascontiguousarray(V_pool[:, c, :].astype(NPBF)),
            "gidx": gidx,
            "cosq": cosqT, "sinq": sinqT, "cosk": coskT, "sink": sinkT,
            "cmask": cm, "rperm": rperm, "identb": identb,
            "selg": selg, "ind2": ind2,
        })
    return in_maps


_NC_CACHE = None


def kernel(**inputs):
    global _NC_CACHE
    in_maps = host_inputs(**inputs)
    if _NC_CACHE is None:
        _NC_CACHE = build_nc()
    res = run_bass_kernel_spmd(_NC_CACHE, in_maps, core_ids=list(range(NCORES)))
    y = res.results[0]["y"].astype(np.float32)
    for c in range(1, NCORES):
        y = y + res.results[c]["y"].astype(np.float32)
    return y.reshape(B, T, C)
